# revision 6
# baseline (speedup 1.0000x reference)
"""AdapterAttention Trainium2 kernel (8 NeuronCores, batch-data-parallel).

Reference computation (per image, N=197 tokens, C=768, 12 heads x 64):
    mid       = tanh(x @ down_w.T + down_b)                  [N, 256]
    kv_prefix = mid @ up_w.T + up_b                          [N, 1536]
    qkv       = x @ qkv_w.T + qkv_b + 0.8*[0, kv_prefix]     [N, 2304]
    attn      = softmax(q k^T / 8) v  per head
    out       = attn_out @ proj_w.T + proj_b

Distribution: batch 64 -> 8 images per core, weights replicated. No
collectives; host shards inputs and reassembles outputs.

On-chip layout strategy (everything "feature-major"/transposed so no
on-chip input transposes are needed; host pre-transposes x and weights):
    xT   [c=768, t=1576]  (t = 8 images * 197 tokens)
    qT/kT[o=768, t]  = W^T-matmuls on xT (+ adapter matmuls for kT)
    V    per image, natural [tok<=128 x 2 chunks, o=768] (lhsT of AV)
    S    natural [i, j] per (image, head) -> exp -> normalize per-partition
    P^T  via PE transpose -> AV gives O^T[d, i] directly -> proj -> outT
The v-bias is folded into proj bias on the host (softmax rows sum to 1);
0.8 prefix scale is folded into up_w/up_b on the host.
"""

import sys

sys.path.insert(0, "/opt/trn_rl_repo")

import ml_dtypes
import numpy as np

DIM = 768
MID = 256
HEADS = 12
D = 64
P_SCALE = 0.8
SCALE = D ** -0.5
B_FULL = 64
N_TOK = 197
N_CORES = 8
B_LOC = B_FULL // N_CORES          # 8 images per core
T = B_LOC * N_TOK                  # 1576 tokens per core
NT = 394                           # t-chunk for dense matmuls (= 2 images)
N_NT = T // NT                     # 4
CC = DIM // 128                    # 6 contraction chunks over c
MC = MID // 128                    # 2 contraction chunks over mid
OT = DIM // 128                    # 6 output tiles over o / co

BF16 = ml_dtypes.bfloat16

_CACHE = {}


def _build():
    from contextlib import ExitStack

    import concourse.tile as tile
    from concourse import bacc, mybir
    from concourse.masks import make_identity

    dt = mybir.dt
    AF = mybir.ActivationFunctionType

    nc = bacc.Bacc(
        "TRN2", target_bir_lowering=False, debug=False, num_devices=N_CORES
    )

    def din(name, shape, dtype):
        return nc.dram_tensor(name, shape, dtype, kind="ExternalInput").ap()

    xT_d = din("xT", [DIM, T], dt.bfloat16)
    wq_d = din("wq", [DIM, DIM], dt.bfloat16)      # [c, o]
    wk_d = din("wk", [DIM, DIM], dt.bfloat16)
    wv_d = din("wv", [DIM, DIM], dt.bfloat16)
    dw_d = din("dw", [DIM, MID], dt.bfloat16)      # down_w.T  [c, m]
    uk_d = din("uk", [MID, DIM], dt.bfloat16)      # 0.8*up_w[:C].T  [m, o]
    uv_d = din("uv", [MID, DIM], dt.bfloat16)      # 0.8*up_w[C:].T  [m, o]
    pw_d = din("pw", [DIM, DIM], dt.bfloat16)      # proj_w.T  [o, co]
    bq_d = din("bq", [128, OT], dt.float32)
    bk_d = din("bk", [128, OT], dt.float32)
    bd_d = din("bd", [128, MC], dt.float32)
    bp_d = din("bp", [128, OT], dt.float32)
    out_d = nc.dram_tensor("outT", [DIM, T], dt.float32, kind="ExternalOutput").ap()

    with tile.TileContext(nc) as tc, ExitStack() as ctx:
        persist = ctx.enter_context(tc.tile_pool(name="persist", bufs=1))

        xT = persist.tile([128, CC, T], dt.bfloat16)
        nc.sync.dma_start(out=xT[:], in_=xT_d.rearrange("(c p) t -> p c t", p=128))
        wq = persist.tile([128, CC, DIM], dt.bfloat16)
        nc.sync.dma_start(out=wq[:], in_=wq_d.rearrange("(c p) o -> p c o", p=128))
        wk = persist.tile([128, CC, DIM], dt.bfloat16)
        nc.sync.dma_start(out=wk[:], in_=wk_d.rearrange("(c p) o -> p c o", p=128))
        wv = persist.tile([128, CC, DIM], dt.bfloat16)
        nc.sync.dma_start(out=wv[:], in_=wv_d.rearrange("(c p) o -> p c o", p=128))
        dw = persist.tile([128, CC, MID], dt.bfloat16)
        nc.sync.dma_start(out=dw[:], in_=dw_d.rearrange("(c p) m -> p c m", p=128))
        uk = persist.tile([128, MC, DIM], dt.bfloat16)
        nc.sync.dma_start(out=uk[:], in_=uk_d.rearrange("(c p) o -> p c o", p=128))
        uv = persist.tile([128, MC, DIM], dt.bfloat16)
        nc.sync.dma_start(out=uv[:], in_=uv_d.rearrange("(c p) o -> p c o", p=128))
        pw = persist.tile([128, OT, DIM], dt.bfloat16)
        nc.sync.dma_start(out=pw[:], in_=pw_d.rearrange("(c p) o -> p c o", p=128))
        bq = persist.tile([128, OT], dt.float32)
        nc.sync.dma_start(out=bq[:], in_=bq_d)
        bk = persist.tile([128, OT], dt.float32)
        nc.sync.dma_start(out=bk[:], in_=bk_d)
        bd = persist.tile([128, MC], dt.float32)
        nc.sync.dma_start(out=bd[:], in_=bd_d)
        bp = persist.tile([128, OT], dt.float32)
        nc.sync.dma_start(out=bp[:], in_=bp_d)

        idt = persist.tile([128, 128], dt.bfloat16)
        make_identity(nc, idt[:])

        qT = persist.tile([128, OT, T], dt.bfloat16)
        kT = persist.tile([128, OT, T], dt.bfloat16)
        OTt = persist.tile([128, OT, T], dt.bfloat16)
        midT = persist.tile([128, MC, T], dt.bfloat16)
        V = [
            persist.tile([128, 2, DIM], dt.bfloat16, tag=f"V{b}", name=f"V{b}")
            for b in range(B_LOC)
        ]

        # ---- Stage A/B: qT, kT, midT, V -------------------------------
        with (
            tc.tile_pool(name="psA", bufs=3, space="PSUM") as pA,
            tc.tile_pool(name="psV", bufs=2, space="PSUM") as pV,
        ):
            for nt in range(N_NT):
                sl = slice(nt * NT, (nt + 1) * NT)
                # midT = tanh(down^T-matmul + bias)
                for mt in range(MC):
                    ps = pA.tile([128, NT], dt.float32, tag="psA")
                    for cc in range(CC):
                        nc.tensor.matmul(
                            ps[:],
                            lhsT=dw[:, cc, mt * 128:(mt + 1) * 128],
                            rhs=xT[:, cc, sl],
                            start=(cc == 0),
                            stop=(cc == CC - 1),
                        )
                    nc.scalar.activation(
                        out=midT[:, mt, sl], in_=ps[:], func=AF.Tanh,
                        bias=bd[:, mt:mt + 1], scale=1.0,
                    )
                # qT
                for ot in range(OT):
                    ps = pA.tile([128, NT], dt.float32, tag="psA")
                    for cc in range(CC):
                        nc.tensor.matmul(
                            ps[:],
                            lhsT=wq[:, cc, ot * 128:(ot + 1) * 128],
                            rhs=xT[:, cc, sl],
                            start=(cc == 0),
                            stop=(cc == CC - 1),
                        )
                    nc.scalar.activation(
                        out=qT[:, ot, sl], in_=ps[:], func=AF.Identity,
                        bias=bq[:, ot:ot + 1], scale=1.0,
                    )
                # kT (x-part + adapter part accumulate into same psum)
                for ot in range(OT):
                    ps = pA.tile([128, NT], dt.float32, tag="psA")
                    for cc in range(CC):
                        nc.tensor.matmul(
                            ps[:],
                            lhsT=wk[:, cc, ot * 128:(ot + 1) * 128],
                            rhs=xT[:, cc, sl],
                            start=(cc == 0),
                            stop=False,
                        )
                    for mc in range(MC):
                        nc.tensor.matmul(
                            ps[:],
                            lhsT=uk[:, mc, ot * 128:(ot + 1) * 128],
                            rhs=midT[:, mc, sl],
                            start=False,
                            stop=(mc == MC - 1),
                        )
                    nc.scalar.activation(
                        out=kT[:, ot, sl], in_=ps[:], func=AF.Identity,
                        bias=bk[:, ot:ot + 1], scale=1.0,
                    )
                # V for the two images inside this t-chunk
                for b in (2 * nt, 2 * nt + 1):
                    t0 = b * N_TOK
                    for jc in range(2):
                        jsz = 128 if jc == 0 else N_TOK - 128
                        tok = slice(t0 + jc * 128, t0 + jc * 128 + jsz)
                        ps = pV.tile([128, DIM], dt.float32, tag="psV")
                        for osl in (slice(0, 512), slice(512, DIM)):
                            for cc in range(CC):
                                nc.tensor.matmul(
                                    ps[:jsz, osl],
                                    lhsT=xT[:, cc, tok],
                                    rhs=wv[:, cc, osl],
                                    start=(cc == 0),
                                    stop=False,
                                )
                            for mc in range(MC):
                                nc.tensor.matmul(
                                    ps[:jsz, osl],
                                    lhsT=midT[:, mc, tok],
                                    rhs=uv[:, mc, osl],
                                    start=False,
                                    stop=(mc == MC - 1),
                                )
                        nc.vector.tensor_copy(out=V[b][:jsz, jc, :], in_=ps[:jsz, :])

        # ---- Stage C: attention per (image, head) ---------------------
        with (
            tc.tile_pool(name="psS", bufs=2, space="PSUM") as pS,
            tc.tile_pool(name="psO", bufs=2, space="PSUM") as pO,
            tc.tile_pool(name="att", bufs=3) as pE,
            tc.tile_pool(name="rsum", bufs=4) as pR,
        ):
            for b in range(B_LOC):
                t0 = b * N_TOK
                for h in range(HEADS):
                    po = 64 * (h % 2)
                    oh = h // 2
                    s_ps = pS.tile([128, 2, N_TOK], dt.float32, tag="s")
                    for ic in range(2):
                        isz = 128 if ic == 0 else N_TOK - 128
                        isl = slice(t0 + ic * 128, t0 + ic * 128 + isz)
                        nc.tensor.matmul(
                            s_ps[:isz, ic, :],
                            lhsT=qT[po:po + 64, oh, isl],
                            rhs=kT[po:po + 64, oh, t0:t0 + N_TOK],
                            start=True,
                            stop=True,
                        )
                    rs = pR.tile([128, 2], dt.float32, tag="rs")
                    rr = pR.tile([128, 2], dt.float32, tag="rr")
                    e_un = pE.tile([128, 2, N_TOK], dt.bfloat16, tag="eun")
                    for ic in range(2):
                        isz = 128 if ic == 0 else N_TOK - 128
                        nc.scalar.activation(
                            out=e_un[:isz, ic, :], in_=s_ps[:isz, ic, :],
                            func=AF.Exp, scale=SCALE,
                            accum_out=rs[:isz, ic:ic + 1],
                        )
                        nc.vector.reciprocal(
                            out=rr[:isz, ic:ic + 1], in_=rs[:isz, ic:ic + 1]
                        )
                    e_nm = pE.tile([128, 2, N_TOK], dt.bfloat16, tag="enm")
                    for ic in range(2):
                        isz = 128 if ic == 0 else N_TOK - 128
                        nc.vector.tensor_scalar_mul(
                            e_nm[:isz, ic, :], e_un[:isz, ic, :],
                            rr[:isz, ic:ic + 1],
                        )
                    # P^T via PE transpose: blocks (ic, jc)
                    # free dim padded to 198 so slot 1 starts 4B-aligned (PSUM)
                    pt_ps = pS.tile([128, 2, 198], dt.bfloat16, tag="pt")
                    for ic in range(2):
                        isz = 128 if ic == 0 else N_TOK - 128
                        for jc in range(2):
                            jsz = 128 if jc == 0 else N_TOK - 128
                            nc.tensor.transpose(
                                out=pt_ps[:jsz, jc, ic * 128:ic * 128 + isz],
                                in_=e_nm[:isz, ic, jc * 128:jc * 128 + jsz],
                                identity=idt[:isz, :isz],
                            )
                    pt_sb = pE.tile([128, 2, N_TOK], dt.bfloat16, tag="ptsb")
                    for jc in range(2):
                        jsz = 128 if jc == 0 else N_TOK - 128
                        nc.vector.tensor_copy(
                            out=pt_sb[:jsz, jc, :], in_=pt_ps[:jsz, jc, :N_TOK]
                        )
                    o_ps = pO.tile([64, N_TOK], dt.float32, tag="o")
                    for jc in range(2):
                        jsz = 128 if jc == 0 else N_TOK - 128
                        nc.tensor.matmul(
                            o_ps[:],
                            lhsT=V[b][:jsz, jc, h * 64:(h + 1) * 64],
                            rhs=pt_sb[:jsz, jc, :],
                            start=(jc == 0),
                            stop=(jc == 1),
                        )
                    nc.vector.tensor_copy(
                        out=OTt[po:po + 64, oh, t0:t0 + N_TOK], in_=o_ps[:]
                    )

        # ---- Stage D: output projection -------------------------------
        with (
            tc.tile_pool(name="psD", bufs=3, space="PSUM") as pD,
            tc.tile_pool(name="stg", bufs=3) as pStg,
        ):
            for nt in range(N_NT):
                sl = slice(nt * NT, (nt + 1) * NT)
                for ct in range(OT):
                    ps = pD.tile([128, NT], dt.float32, tag="d")
                    for oc in range(OT):
                        nc.tensor.matmul(
                            ps[:],
                            lhsT=pw[:, oc, ct * 128:(ct + 1) * 128],
                            rhs=OTt[:, oc, sl],
                            start=(oc == 0),
                            stop=(oc == OT - 1),
                        )
                    st = pStg.tile([128, NT], dt.float32, tag="st")
                    nc.scalar.activation(
                        out=st[:], in_=ps[:], func=AF.Identity,
                        bias=bp[:, ct:ct + 1], scale=1.0,
                    )
                    nc.sync.dma_start(
                        out=out_d[ct * 128:(ct + 1) * 128, sl], in_=st[:]
                    )

    nc.compile()
    return nc


def _prep_inputs(x, qkv_w, qkv_b, proj_w, proj_b, down_w, down_b, up_w, up_b):
    f32 = np.float32
    x = np.asarray(x, f32)
    qkv_w = np.asarray(qkv_w, f32)
    qkv_b = np.asarray(qkv_b, f32)
    proj_w = np.asarray(proj_w, f32)
    proj_b = np.asarray(proj_b, f32)
    down_w = np.asarray(down_w, f32)
    down_b = np.asarray(down_b, f32)
    up_w = np.asarray(up_w, f32)
    up_b = np.asarray(up_b, f32)

    wq = qkv_w[0:DIM]
    wk = qkv_w[DIM:2 * DIM]
    wv = qkv_w[2 * DIM:3 * DIM]
    bq = qkv_b[0:DIM]
    bk = qkv_b[DIM:2 * DIM] + P_SCALE * up_b[0:DIM]
    bv = qkv_b[2 * DIM:3 * DIM] + P_SCALE * up_b[DIM:2 * DIM]
    # v-bias rides through the softmax average unchanged -> fold into proj_b
    bp = proj_b + proj_w @ bv

    def t_bf16(a):
        return np.ascontiguousarray(a.T).astype(BF16)

    def b_lay(vec, nt):
        return np.ascontiguousarray(vec.reshape(nt, 128).T).astype(f32)

    common = {
        "wq": t_bf16(wq),
        "wk": t_bf16(wk),
        "wv": t_bf16(wv),
        "dw": t_bf16(down_w),
        "uk": t_bf16(P_SCALE * up_w[0:DIM]),
        "uv": t_bf16(P_SCALE * up_w[DIM:2 * DIM]),
        "pw": t_bf16(proj_w),
        "bq": b_lay(bq, OT),
        "bk": b_lay(bk, OT),
        "bd": b_lay(down_b, MC),
        "bp": b_lay(bp, OT),
    }
    in_maps = []
    for c in range(N_CORES):
        xc = x[c * B_LOC:(c + 1) * B_LOC].reshape(T, DIM)
        m = dict(common)
        m["xT"] = np.ascontiguousarray(xc.T).astype(BF16)
        in_maps.append(m)
    return in_maps


def kernel(x, qkv_w, qkv_b, proj_w, proj_b, down_w, down_b, up_w, up_b):
    from concourse.bass_utils import run_bass_kernel_spmd

    if "nc" not in _CACHE:
        _CACHE["nc"] = _build()
    nc = _CACHE["nc"]

    in_maps = _prep_inputs(
        x, qkv_w, qkv_b, proj_w, proj_b, down_w, down_b, up_w, up_b
    )
    res = run_bass_kernel_spmd(nc, in_maps, list(range(N_CORES)))
    outs = []
    for i in range(N_CORES):
        oT = np.asarray(res.results[i]["outT"], dtype=np.float32)
        outs.append(np.ascontiguousarray(oT.T).reshape(B_LOC, N_TOK, DIM))
    return np.concatenate(outs, axis=0)
